# revision 10
# baseline (speedup 1.0000x reference)
"""Adaptive mean thresholding (11x11 box, replicate border, C=0.02) on 8
TRN2 NeuronCores. Batch [128,512,512] f32 -> binary-inv threshold map.

v2.1 design: balance DVE/ACT/PE/GpSimd/DMA near the DMA roofline.

  - u8 output (4x store-traffic cut), plane-major per row
    [evens(256)|odds(256)]; host re-interleaves + casts to f32. Stores
    batched 4 images -> 1 MB SWDGE transfers.
  - DVE: pair sums qb (fp16 2x), ONE scan per image extended 6 steps so
    the initial value is just KAPPA (no ACT-side init/seed ops; qb tile
    has a gpsimd-zeroed 6-elem prefix), and the overcount presubtract
    W = R6 - x as two FULL-WIDTH contiguous fp16 2x tensor_tensors
    (6 junk elems per block are computed and never read - contiguity
    keeps the 2x packed mode, unlike block-strided 3D views).
  - TensorE: positive band passes on W only (20 MM) + the -121*I fold
    of the compare operand x into PSUM (8 MM): z = box - 121C - 121x.
  - ACT: deinterleave x into fp16 even/odd planes + the entire compare:
    u8 out = Sign(z) straight from PSUM (f32->u8 saturates -1 -> 0).
  - GpSimd: replicate pads, qb zero-prefix memset, batched store issue.
"""

import numpy as np

B, H, W = 128, 512, 512
NCORES = 8
NIMG = B // NCORES          # 16 images per core
P = 128                     # partitions
NB = H // P                 # 4 row blocks per image
K = 11                      # box size
PAD = 6                     # left/right replicate pads per block
BW = PAD + W + PAD          # 524: one padded block
IW = NB * BW                # 2096: one padded image
NQ = BW // 2                # 262 pairs per block
NQI = NB * NQ               # 1048 pairs per image
MQ = W // 2                 # 256 output columns per plane per block
KAPPA = -2.42 / K           # folds "- 121*C" into the scan init

_CACHE = {}


def _band_weights():
    """512x512 vertical box-filter count matrix, sliced to the five
    distinct 128x128 blocks, plus the -121*I stationary for the x fold."""
    Bm = np.zeros((H, H), dtype=np.float32)
    for i in range(H):
        for d in range(-5, 6):
            r = min(max(i + d, 0), H - 1)
            Bm[r, i] += 1.0
    W0 = Bm[0:128, 0:128]        # block 0 main (top replicate folded)
    WI = Bm[128:256, 128:256]    # interior main (pure band)
    W3 = Bm[384:512, 384:512]    # block 3 main (bottom replicate folded)
    WDN = Bm[0:128, 128:256]     # contribution from tile I-1 to block I
    WUP = Bm[128:256, 0:128]     # contribution from tile I+1 to block I
    DIAG = -121.0 * np.eye(128, dtype=np.float32)
    return np.ascontiguousarray(np.stack(
        [W0, WI, W3, WDN, WUP, -WI, -W3, -WDN, -WUP, DIAG]
    )).astype(np.float16)


def _build():
    import concourse.bass as bass  # noqa: F401
    import concourse.tile as tile
    from concourse import bacc, mybir
    from concourse.alu_op_type import AluOpType

    F32 = mybir.dt.float32
    F16 = mybir.dt.float16
    U8 = mybir.dt.uint8
    ACT_COPY = mybir.ActivationFunctionType.Copy
    ACT_SIGN = mybir.ActivationFunctionType.Sign

    nc = bacc.Bacc("TRN2", target_bir_lowering=False, debug=False,
                   num_devices=NCORES)
    in_ext = nc.dram_tensor("input", [NIMG, H, W], F32,
                            kind="ExternalInput").ap()
    wts_ext = nc.dram_tensor("wts", [10, 128, 128], F16,
                             kind="ExternalInput").ap()
    out_ext = nc.dram_tensor("output", [NIMG, H, W], U8,
                             kind="ExternalOutput").ap()

    with tile.TileContext(nc) as tc:
        with tc.tile_pool(name="consts", bufs=1) as consts, \
             tc.tile_pool(name="xp", bufs=4) as xp_pool, \
             tc.tile_pool(name="xq", bufs=4) as xq_pool, \
             tc.tile_pool(name="qb", bufs=4) as qb_pool, \
             tc.tile_pool(name="r6", bufs=4) as r6_pool, \
             tc.tile_pool(name="wq", bufs=4) as wq_pool, \
             tc.tile_pool(name="ot", bufs=2) as ot_pool, \
             tc.tile_pool(name="psum", bufs=2, space="PSUM") as psum:

            wt = consts.tile([P, 10 * 128], F16)
            wv = wt[:].rearrange("p (s m) -> p s m", s=10)
            (W0, WI, W3, WDN, WUP,
             NI, N3, NDN, NUP, DIAG) = (wv[:, s, :] for s in range(10))
            MAIN = (W0, WI, WI, W3)
            NMAIN = (None, None, NI, N3)
            wts_loaded = []

            def load_wts():
                if not wts_loaded:
                    nc.gpsimd.dma_start(
                        wt[:].rearrange("p (s m) -> p s m", s=10),
                        wts_ext.rearrange("s p m -> p s m"))
                    wts_loaded.append(True)

            live = {}
            otbufs = {}

            def front(im):
                xp = xp_pool.tile([P, IW], F32, tag="xp")
                xpv = xp[:].rearrange("p (b c) -> p b c", b=NB)
                src = in_ext[im].rearrange("(b p) w -> p b w", p=P)
                nc.sync.dma_start(xpv[:, :, PAD:PAD + W], src)
                # replicate-pad block edges (GpSimd; tiny broadcast copies)
                nc.gpsimd.tensor_copy(
                    xpv[:, :, 0:PAD],
                    xpv[:, :, PAD:PAD + 1].broadcast_to([P, NB, PAD]))
                nc.gpsimd.tensor_copy(
                    xpv[:, :, PAD + W:BW],
                    xpv[:, :, PAD + W - 1:PAD + W]
                    .broadcast_to([P, NB, PAD]))
                # deinterleave to fp16 even/odd planes (ACT)
                # (+6 tail pad so full-width shifted views stay in range)
                xq = xq_pool.tile([P, 2 * NQI + PAD], F16, tag="xq")
                xqe = xq[:, 0:NQI]
                xqo = xq[:, NQI:2 * NQI]
                xpar = xp[:].rearrange("p (b m r) -> p r b m", b=NB, r=2)
                nc.scalar.activation(
                    xq[:, 0:2 * NQI].rearrange(
                        "p (r b m) -> p r b m", r=2, b=NB),
                    xpar, ACT_COPY, bias=0.0, scale=1.0)
                # pair sums qb (DVE fp16 2x) into a zero-prefixed buffer
                qbz = qb_pool.tile([P, PAD + NQI], F16, tag="qb")
                nc.gpsimd.memset(qbz[:, 0:PAD], 0)
                nc.vector.tensor_tensor(
                    out=qbz[:, PAD:], in0=xqe, in1=xqo, op=AluOpType.add)
                # 6-tap sliding sum over the pair domain (DVE scan),
                # extended 6 steps so initial is just KAPPA:
                #   sh6n[t] = sum(qb[t-5..t]) + KAPPA   (valid for t >= 5)
                # old-style R6 starting at pair m == sh6n[m+5].
                sh6 = r6_pool.tile([P, NQI + PAD], F16, tag="sh6")
                nc.vector.tensor_tensor_scan(
                    out=sh6[:, 0:NQI],
                    data0=qbz[:, PAD:PAD + NQI],
                    data1=qbz[:, 0:NQI],
                    initial=KAPPA,
                    op0=AluOpType.add, op1=AluOpType.subtract)

                # presubtract (DVE fp16 2x, full-width contiguous):
                #   we[j] = sh6n[j+5] - xqe[j]   (= R6[j] - x_even[j])
                #   wo[j] = sh6n[j+6] - xqo[j+6] (= R6[j+1] - x_odd[j+6])
                # junk at j%262 >= 256 is never consumed by the matmuls.
                # wo only covers source blocks 0-2 (odd planes of output
                # blocks 0-1); blocks 2-3's odd planes are fixed on the
                # TensorE instead (neg-band matmuls) to unload the DVE.
                WOL = 3 * NQ
                wq = wq_pool.tile([P, NQI + WOL], F16, tag="wq")
                we = wq[:, 0:NQI]
                wo = wq[:, NQI:NQI + WOL]
                nc.vector.tensor_tensor(
                    out=we, in0=sh6[:, 5:5 + NQI], in1=xq[:, 0:NQI],
                    op=AluOpType.subtract)
                nc.vector.tensor_tensor(
                    out=wo, in0=sh6[:, 6:6 + WOL],
                    in1=xq[:, NQI + PAD:NQI + PAD + WOL],
                    op=AluOpType.subtract)

                # vertical band matmuls: positive band on W planes, then
                # -121*I on the compare-x slices so PSUM holds
                # z = box - 121*C - 121*x
                load_wts()
                ps = psum.tile([P, NB * W], F32, tag="ps")
                for b in range(NB):
                    lo = max(b - 1, 0)
                    hi = min(b + 1, NB - 1)
                    srcs = []
                    for t in range(lo, hi + 1):
                        wmat = MAIN[b] if t == b else \
                               (WDN if t == b - 1 else WUP)
                        srcs.append((t, wmat))
                    for pl in range(2):
                        psl = ps[:, W * b + MQ * pl:W * b + MQ * (pl + 1)]
                        i = 0
                        if pl == 0 or b < 2:
                            wpl = we if pl == 0 else wo
                            for t, wmat in srcs:
                                nc.tensor.matmul(
                                    psl, wmat,
                                    wpl[:, NQ * t:NQ * t + MQ],
                                    start=(i == 0), stop=False)
                                i += 1
                        else:
                            # odd plane, blocks 2-3: positive band on the
                            # raw scan + negative band on the odd x plane
                            for t, wmat in srcs:
                                nc.tensor.matmul(
                                    psl, wmat,
                                    sh6[:, 6 + NQ * t:6 + NQ * t + MQ],
                                    start=(i == 0), stop=False)
                                i += 1
                            for t, _ in srcs:
                                nmat = NMAIN[b] if t == b else \
                                       (NDN if t == b - 1 else NUP)
                                nc.tensor.matmul(
                                    psl, nmat,
                                    xq[:, NQI + PAD + NQ * t:
                                       NQI + PAD + NQ * t + MQ],
                                    start=False, stop=False)
                        xoff = (0 if pl == 0 else NQI) + NQ * b + 3
                        nc.tensor.matmul(psl, DIAG,
                                         xq[:, xoff:xoff + MQ],
                                         start=False, stop=True)
                live[im] = (ps,)

            def compare(im, ps, ot, slot):
                """Sign(z) -> u8 into slot of a 4-image store buffer."""
                nc.scalar.activation(
                    ot[:, slot * NB * W:(slot + 1) * NB * W],
                    ps[:], ACT_SIGN, bias=0.0, scale=1.0)

            def store_batch(i0, n, eng):
                ot = otbufs.pop(i0)
                dst = out_ext[i0:i0 + n].rearrange(
                    "i (b p) w -> p i b w", p=P)
                eng.dma_start(
                    dst, ot[:, 0:n * NB * W].rearrange(
                        "p (i b w) -> p i b w", i=n, b=NB))

            def epilogue(im):
                (ps,) = live.pop(im)
                if im < 12:
                    i0 = im - im % 4
                    if im % 4 == 0:
                        otbufs[i0] = ot_pool.tile(
                            [P, 4 * NB * W], U8, tag="ot4", name="ot4")
                    compare(im, ps, otbufs[i0], im % 4)
                    if im % 4 == 3:
                        store_batch(i0, 4, nc.gpsimd)
                else:
                    # tail: per-image stores to keep the drain short
                    otbufs[im] = ot_pool.tile(
                        [P, 4 * NB * W], U8, tag="ot4", name="ot1")
                    compare(im, ps, otbufs[im], 0)
                    store_batch(im, 1,
                                nc.gpsimd if im < 14 else nc.sync)

            for im in range(NIMG):
                front(im)
                if im >= 1:
                    epilogue(im - 1)
            epilogue(NIMG - 1)

    nc.compile()
    return nc


def _get_nc():
    if "nc" not in _CACHE:
        _CACHE["nc"] = _build()
        _CACHE["wts"] = _band_weights()
    return _CACHE["nc"]


def postprocess(outputs) -> np.ndarray:
    """Device output is u8 with each row stored plane-major
    [evens(256) | odds(256)]; re-interleave and cast to f32."""
    raw = np.concatenate(list(outputs), axis=0)          # [B,H,W] u8
    v = raw.reshape(B, H, 2, W // 2)
    out = np.empty((B, H, W), dtype=np.float32)
    out[:, :, 0::2] = v[:, :, 0]
    out[:, :, 1::2] = v[:, :, 1]
    return out


def kernel(input_batch: np.ndarray) -> np.ndarray:
    from concourse.bass_utils import run_bass_kernel_spmd

    nc = _get_nc()
    wts = _CACHE["wts"]
    assert input_batch.shape == (B, H, W)
    x = np.ascontiguousarray(input_batch, dtype=np.float32)
    in_maps = [
        {"input": x[c * NIMG:(c + 1) * NIMG], "wts": wts}
        for c in range(NCORES)
    ]
    res = run_bass_kernel_spmd(nc, in_maps, core_ids=list(range(NCORES)))
    return postprocess(r["output"] for r in res.results)


if __name__ == "__main__":
    rng = np.random.default_rng(0)
    x = rng.random((B, H, W), dtype=np.float32)
    y = kernel(x)
    print(y.shape, y.dtype, y.mean())


# revision 15
# speedup vs baseline: 1.0160x; 1.0160x over previous
"""Adaptive mean thresholding (11x11 box, replicate border, C=0.02) on 8
TRN2 NeuronCores. Batch [128,512,512] f32 -> binary-inv threshold map.

v2.1 design: balance DVE/ACT/PE/GpSimd/DMA near the DMA roofline.

  - u8 output (4x store-traffic cut), plane-major per row
    [evens(256)|odds(256)]; host re-interleaves + casts to f32. Stores
    batched 4 images -> 1 MB SWDGE transfers.
  - DVE: pair sums qb (fp16 2x), ONE scan per image extended 6 steps so
    the initial value is just KAPPA (no ACT-side init/seed ops; qb tile
    has a gpsimd-zeroed 6-elem prefix), and the overcount presubtract
    W = R6 - x as two FULL-WIDTH contiguous fp16 2x tensor_tensors
    (6 junk elems per block are computed and never read - contiguity
    keeps the 2x packed mode, unlike block-strided 3D views).
  - TensorE: positive band passes on W only (20 MM) + the -121*I fold
    of the compare operand x into PSUM (8 MM): z = box - 121C - 121x.
  - ACT: deinterleave x into fp16 even/odd planes + the entire compare:
    u8 out = Sign(z) straight from PSUM (f32->u8 saturates -1 -> 0).
  - GpSimd: replicate pads, qb zero-prefix memset, batched store issue.
"""

import numpy as np

B, H, W = 128, 512, 512
NCORES = 8
NIMG = B // NCORES          # 16 images per core
P = 128                     # partitions
NB = H // P                 # 4 row blocks per image
K = 11                      # box size
PAD = 6                     # left/right replicate pads per block
BW = PAD + W + PAD          # 524: one padded block
IW = NB * BW                # 2096: one padded image
NQ = BW // 2                # 262 pairs per block
NQI = NB * NQ               # 1048 pairs per image
MQ = W // 2                 # 256 output columns per plane per block
KAPPA = -2.42 / K           # folds "- 121*C" into the scan init

_CACHE = {}


def _band_weights():
    """512x512 vertical box-filter count matrix, sliced to the five
    distinct 128x128 blocks, plus the -121*I stationary for the x fold."""
    Bm = np.zeros((H, H), dtype=np.float32)
    for i in range(H):
        for d in range(-5, 6):
            r = min(max(i + d, 0), H - 1)
            Bm[r, i] += 1.0
    W0 = Bm[0:128, 0:128]        # block 0 main (top replicate folded)
    WI = Bm[128:256, 128:256]    # interior main (pure band)
    W3 = Bm[384:512, 384:512]    # block 3 main (bottom replicate folded)
    WDN = Bm[0:128, 128:256]     # contribution from tile I-1 to block I
    WUP = Bm[128:256, 0:128]     # contribution from tile I+1 to block I
    DIAG = -121.0 * np.eye(128, dtype=np.float32)
    return np.ascontiguousarray(
        np.stack([W0, WI, W3, WDN, WUP, DIAG])).astype(np.float16)


def _build():
    import concourse.bass as bass  # noqa: F401
    import concourse.tile as tile
    from concourse import bacc, mybir
    from concourse.alu_op_type import AluOpType

    F32 = mybir.dt.float32
    F16 = mybir.dt.float16
    U8 = mybir.dt.uint8
    ACT_COPY = mybir.ActivationFunctionType.Copy
    ACT_SIGN = mybir.ActivationFunctionType.Sign

    nc = bacc.Bacc("TRN2", target_bir_lowering=False, debug=False,
                   num_devices=NCORES)
    in_ext = nc.dram_tensor("input", [NIMG, H, W], F32,
                            kind="ExternalInput").ap()
    wts_ext = nc.dram_tensor("wts", [6, 128, 128], F16,
                             kind="ExternalInput").ap()
    out_ext = nc.dram_tensor("output", [NIMG, H, W], U8,
                             kind="ExternalOutput").ap()

    with tile.TileContext(nc) as tc:
        with tc.tile_pool(name="consts", bufs=1) as consts, \
             tc.tile_pool(name="xp", bufs=4) as xp_pool, \
             tc.tile_pool(name="xq", bufs=4) as xq_pool, \
             tc.tile_pool(name="qb", bufs=4) as qb_pool, \
             tc.tile_pool(name="r6", bufs=4) as r6_pool, \
             tc.tile_pool(name="wq", bufs=4) as wq_pool, \
             tc.tile_pool(name="ot", bufs=2) as ot_pool, \
             tc.tile_pool(name="psum", bufs=2, space="PSUM") as psum:

            wt = consts.tile([P, 6 * 128], F16)
            wv = wt[:].rearrange("p (s m) -> p s m", s=6)
            W0, WI, W3, WDN, WUP, DIAG = (wv[:, s, :] for s in range(6))
            MAIN = (W0, WI, WI, W3)
            wts_loaded = []

            def load_wts():
                if not wts_loaded:
                    nc.gpsimd.dma_start(
                        wt[:].rearrange("p (s m) -> p s m", s=6),
                        wts_ext.rearrange("s p m -> p s m"))
                    wts_loaded.append(True)

            live = {}
            otbufs = {}

            NQ2 = 2 * NQI       # 2096 pairs per image pair

            def front(im, xq):
                """Per-image load + pads + deinterleave into half of the
                pair-shared xq buffer (layout: [e0|e1|o0|o1|pad])."""
                k = im % 2
                xp = xp_pool.tile([P, IW], F32, tag="xp")
                xpv = xp[:].rearrange("p (b c) -> p b c", b=NB)
                src = in_ext[im].rearrange("(b p) w -> p b w", p=P)
                nc.sync.dma_start(xpv[:, :, PAD:PAD + W], src)
                # replicate-pad block edges (GpSimd; tiny broadcast copies)
                nc.gpsimd.tensor_copy(
                    xpv[:, :, 0:PAD],
                    xpv[:, :, PAD:PAD + 1].broadcast_to([P, NB, PAD]))
                nc.gpsimd.tensor_copy(
                    xpv[:, :, PAD + W:BW],
                    xpv[:, :, PAD + W - 1:PAD + W]
                    .broadcast_to([P, NB, PAD]))
                # deinterleave to fp16 even/odd planes (ACT): one op
                # writing this image's even half and odd half
                xpar = xp[:].rearrange("p (b m r) -> p r b m", b=NB, r=2)
                nc.scalar.activation(
                    xq[:, 0:2 * NQ2].rearrange(
                        "p (r i b m) -> p i r b m", r=2, i=2, b=NB)[:, k],
                    xpar, ACT_COPY, bias=0.0, scale=1.0)

            def dve_pair(xq):
                """Paired DVE stages: pair sums, ONE scan spanning both
                images (the 6-tap subtract recurrence self-corrects
                across image seams exactly as across block seams), and
                full-width presubtracts. Returns (sh6, wq)."""
                qbz = qb_pool.tile([P, PAD + NQ2], F16, tag="qb")
                nc.gpsimd.memset(qbz[:, 0:PAD], 0)
                nc.vector.tensor_tensor(
                    out=qbz[:, PAD:], in0=xq[:, 0:NQ2],
                    in1=xq[:, NQ2:2 * NQ2], op=AluOpType.add)
                sh6 = r6_pool.tile([P, NQ2 + PAD], F16, tag="sh6")
                nc.vector.tensor_tensor_scan(
                    out=sh6[:, 0:NQ2],
                    data0=qbz[:, PAD:PAD + NQ2],
                    data1=qbz[:, 0:NQ2],
                    initial=KAPPA,
                    op0=AluOpType.add, op1=AluOpType.subtract)
                # presubtract (DVE fp16 2x, full-width contiguous):
                #   we[j] = sh6n[j+5] - xqe[j]; wo[j] = sh6n[j+6] - xqo[j+6]
                wq = wq_pool.tile([P, 2 * NQ2], F16, tag="wq")
                nc.vector.tensor_tensor(
                    out=wq[:, 0:NQ2], in0=sh6[:, 5:5 + NQ2],
                    in1=xq[:, 0:NQ2], op=AluOpType.subtract)
                nc.vector.tensor_tensor(
                    out=wq[:, NQ2:], in0=sh6[:, 6:6 + NQ2],
                    in1=xq[:, NQ2 + PAD:2 * NQ2 + PAD],
                    op=AluOpType.subtract)
                return sh6, wq

                # vertical band matmuls: positive band on W planes, then
            def mm_image(im, xq, wq):
                """Band matmuls for image im (k-th in its pair): positive
                band on the presubtracted planes + -121*I on the
                compare-x slices so PSUM holds z = box - 121*C - 121*x."""
                k = im % 2
                load_wts()
                ps = psum.tile([P, NB * W], F32, tag="ps")
                for b in range(NB):
                    lo = max(b - 1, 0)
                    hi = min(b + 1, NB - 1)
                    srcs = []
                    for t in range(lo, hi + 1):
                        wmat = MAIN[b] if t == b else \
                               (WDN if t == b - 1 else WUP)
                        srcs.append((t, wmat))
                    for pl in range(2):
                        psl = ps[:, W * b + MQ * pl:W * b + MQ * (pl + 1)]
                        wbase = pl * NQ2 + k * NQI
                        i = 0
                        for t, wmat in srcs:
                            off = wbase + NQ * t
                            nc.tensor.matmul(psl, wmat,
                                             wq[:, off:off + MQ],
                                             start=(i == 0), stop=False)
                            i += 1
                        xoff = pl * NQ2 + k * NQI + NQ * b + 3
                        nc.tensor.matmul(psl, DIAG,
                                         xq[:, xoff:xoff + MQ],
                                         start=False, stop=True)
                live[im] = (ps,)

            def compare(im, ps, ot, slot):
                """Sign(z) -> u8 into slot of a 4-image store buffer."""
                nc.scalar.activation(
                    ot[:, slot * NB * W:(slot + 1) * NB * W],
                    ps[:], ACT_SIGN, bias=0.0, scale=1.0)

            def store_batch(i0, n, eng):
                ot = otbufs.pop(i0)
                dst = out_ext[i0:i0 + n].rearrange(
                    "i (b p) w -> p i b w", p=P)
                eng.dma_start(
                    dst, ot[:, 0:n * NB * W].rearrange(
                        "p (i b w) -> p i b w", i=n, b=NB))

            def epilogue(im):
                (ps,) = live.pop(im)
                if im < 12:
                    i0 = im - im % 4
                    if im % 4 == 0:
                        otbufs[i0] = ot_pool.tile(
                            [P, 4 * NB * W], U8, tag="ot4", name="ot4")
                    compare(im, ps, otbufs[i0], im % 4)
                    if im % 4 == 3:
                        store_batch(i0, 4, nc.gpsimd)
                else:
                    # tail: per-image stores to keep the drain short;
                    # the last image's compare is split in half so it
                    # overlaps the trailing matmul blocks
                    otbufs[im] = ot_pool.tile(
                        [P, 4 * NB * W], U8, tag="ot4", name="ot1")
                    if im == NIMG - 1:
                        hw = NB * W // 2
                        nc.scalar.activation(
                            otbufs[im][:, 0:hw], ps[:, 0:hw],
                            ACT_SIGN, bias=0.0, scale=1.0)
                        nc.scalar.activation(
                            otbufs[im][:, hw:NB * W], ps[:, hw:NB * W],
                            ACT_SIGN, bias=0.0, scale=1.0)
                    else:
                        compare(im, ps, otbufs[im], 0)
                    store_batch(im, 1,
                                nc.gpsimd if im < 14 else nc.sync)

            for p in range(NIMG // 2):
                xq = xq_pool.tile([P, 2 * NQ2 + PAD], F16, tag="xq")
                front(2 * p, xq)
                front(2 * p + 1, xq)
                sh6, wq = dve_pair(xq)
                if p >= 1:
                    epilogue(2 * p - 2)
                    epilogue(2 * p - 1)
                mm_image(2 * p, xq, wq)
                mm_image(2 * p + 1, xq, wq)
            epilogue(NIMG - 2)
            epilogue(NIMG - 1)

    nc.compile()
    return nc


def _get_nc():
    if "nc" not in _CACHE:
        _CACHE["nc"] = _build()
        _CACHE["wts"] = _band_weights()
    return _CACHE["nc"]


def postprocess(outputs) -> np.ndarray:
    """Device output is u8 with each row stored plane-major
    [evens(256) | odds(256)]; re-interleave and cast to f32."""
    raw = np.concatenate(list(outputs), axis=0)          # [B,H,W] u8
    v = raw.reshape(B, H, 2, W // 2)
    out = np.empty((B, H, W), dtype=np.float32)
    out[:, :, 0::2] = v[:, :, 0]
    out[:, :, 1::2] = v[:, :, 1]
    return out


def kernel(input_batch: np.ndarray) -> np.ndarray:
    from concourse.bass_utils import run_bass_kernel_spmd

    nc = _get_nc()
    wts = _CACHE["wts"]
    assert input_batch.shape == (B, H, W)
    x = np.ascontiguousarray(input_batch, dtype=np.float32)
    in_maps = [
        {"input": x[c * NIMG:(c + 1) * NIMG], "wts": wts}
        for c in range(NCORES)
    ]
    res = run_bass_kernel_spmd(nc, in_maps, core_ids=list(range(NCORES)))
    return postprocess(r["output"] for r in res.results)


if __name__ == "__main__":
    rng = np.random.default_rng(0)
    x = rng.random((B, H, W), dtype=np.float32)
    y = kernel(x)
    print(y.shape, y.dtype, y.mean())


# revision 16
# speedup vs baseline: 1.1540x; 1.1359x over previous
"""Adaptive mean thresholding (11x11 box, replicate border, C=0.02) on 8
TRN2 NeuronCores. Batch [128,512,512] f32 -> binary-inv threshold map.

v2.1 design: balance DVE/ACT/PE/GpSimd/DMA near the DMA roofline.

  - u8 output (4x store-traffic cut), plane-major per row
    [evens(256)|odds(256)]; host re-interleaves + casts to f32. Stores
    batched 4 images -> 1 MB SWDGE transfers.
  - DVE: pair sums qb (fp16 2x), ONE scan per image extended 6 steps so
    the initial value is just KAPPA (no ACT-side init/seed ops; qb tile
    has a gpsimd-zeroed 6-elem prefix), and the overcount presubtract
    W = R6 - x as two FULL-WIDTH contiguous fp16 2x tensor_tensors
    (6 junk elems per block are computed and never read - contiguity
    keeps the 2x packed mode, unlike block-strided 3D views).
  - TensorE: positive band passes on W only (20 MM) + the -121*I fold
    of the compare operand x into PSUM (8 MM): z = box - 121C - 121x.
  - ACT: deinterleave x into fp16 even/odd planes + the entire compare:
    u8 out = Sign(z) straight from PSUM (f32->u8 saturates -1 -> 0).
  - GpSimd: replicate pads, qb zero-prefix memset, batched store issue.
"""

import numpy as np

B, H, W = 128, 512, 512
NCORES = 8
NIMG = B // NCORES          # 16 images per core
P = 128                     # partitions
NB = H // P                 # 4 row blocks per image
K = 11                      # box size
PAD = 6                     # left/right replicate pads per block
BW = PAD + W + PAD          # 524: one padded block
IW = NB * BW                # 2096: one padded image
NQ = BW // 2                # 262 pairs per block
NQI = NB * NQ               # 1048 pairs per image
MQ = W // 2                 # 256 output columns per plane per block
KAPPA = -2.42 / K           # folds "- 121*C" into the scan init

_CACHE = {}


def _band_weights():
    """512x512 vertical box-filter count matrix, sliced to the five
    distinct 128x128 blocks, plus the -121*I stationary for the x fold."""
    Bm = np.zeros((H, H), dtype=np.float32)
    for i in range(H):
        for d in range(-5, 6):
            r = min(max(i + d, 0), H - 1)
            Bm[r, i] += 1.0
    W0 = Bm[0:128, 0:128]        # block 0 main (top replicate folded)
    WI = Bm[128:256, 128:256]    # interior main (pure band)
    W3 = Bm[384:512, 384:512]    # block 3 main (bottom replicate folded)
    WDN = Bm[0:128, 128:256]     # contribution from tile I-1 to block I
    WUP = Bm[128:256, 0:128]     # contribution from tile I+1 to block I
    DIAG = -121.0 * np.eye(128, dtype=np.float32)
    return np.ascontiguousarray(
        np.stack([W0, WI, W3, WDN, WUP, DIAG])).astype(np.float16)


def _build():
    import concourse.bass as bass  # noqa: F401
    import concourse.tile as tile
    from concourse import bacc, mybir
    from concourse.alu_op_type import AluOpType

    F32 = mybir.dt.float32
    F16 = mybir.dt.float16
    U8 = mybir.dt.uint8
    ACT_COPY = mybir.ActivationFunctionType.Copy
    ACT_SIGN = mybir.ActivationFunctionType.Sign

    nc = bacc.Bacc("TRN2", target_bir_lowering=False, debug=False,
                   num_devices=NCORES)
    in_ext = nc.dram_tensor("input", [NIMG, H, W], F32,
                            kind="ExternalInput").ap()
    wts_ext = nc.dram_tensor("wts", [6, 128, 128], F16,
                             kind="ExternalInput").ap()
    out_ext = nc.dram_tensor("output", [NIMG, H, W], U8,
                             kind="ExternalOutput").ap()

    with tile.TileContext(nc) as tc:
        with tc.tile_pool(name="consts", bufs=1) as consts, \
             tc.tile_pool(name="xp", bufs=4) as xp_pool, \
             tc.tile_pool(name="xq", bufs=4) as xq_pool, \
             tc.tile_pool(name="qb", bufs=4) as qb_pool, \
             tc.tile_pool(name="r6", bufs=4) as r6_pool, \
             tc.tile_pool(name="wq", bufs=4) as wq_pool, \
             tc.tile_pool(name="ot", bufs=2) as ot_pool, \
             tc.tile_pool(name="psum", bufs=2, space="PSUM") as psum:

            wt = consts.tile([P, 6 * 128], F16)
            wv = wt[:].rearrange("p (s m) -> p s m", s=6)
            W0, WI, W3, WDN, WUP, DIAG = (wv[:, s, :] for s in range(6))
            MAIN = (W0, WI, WI, W3)
            wts_loaded = []

            def load_wts():
                if not wts_loaded:
                    nc.gpsimd.dma_start(
                        wt[:].rearrange("p (s m) -> p s m", s=6),
                        wts_ext.rearrange("s p m -> p s m"))
                    wts_loaded.append(True)

            live = {}
            otbufs = {}

            def front(im):
                xp = xp_pool.tile([P, IW], F32, tag="xp")
                xpv = xp[:].rearrange("p (b c) -> p b c", b=NB)
                src = in_ext[im].rearrange("(b p) w -> p b w", p=P)
                nc.sync.dma_start(xpv[:, :, PAD:PAD + W], src)
                # replicate-pad block edges (GpSimd; tiny broadcast copies)
                nc.gpsimd.tensor_copy(
                    xpv[:, :, 0:PAD],
                    xpv[:, :, PAD:PAD + 1].broadcast_to([P, NB, PAD]))
                nc.gpsimd.tensor_copy(
                    xpv[:, :, PAD + W:BW],
                    xpv[:, :, PAD + W - 1:PAD + W]
                    .broadcast_to([P, NB, PAD]))
                # deinterleave to fp16 even/odd planes (ACT)
                # (+6 tail pad so full-width shifted views stay in range)
                xq = xq_pool.tile([P, 2 * NQI + PAD], F16, tag="xq")
                xqe = xq[:, 0:NQI]
                xqo = xq[:, NQI:2 * NQI]
                xpar = xp[:].rearrange("p (b m r) -> p r b m", b=NB, r=2)
                nc.scalar.activation(
                    xq[:, 0:2 * NQI].rearrange(
                        "p (r b m) -> p r b m", r=2, b=NB),
                    xpar, ACT_COPY, bias=0.0, scale=1.0)
                # pair sums qb (DVE fp16 2x) into a zero-prefixed buffer
                qbz = qb_pool.tile([P, PAD + NQI], F16, tag="qb")
                nc.gpsimd.memset(qbz[:, 0:PAD], 0)
                nc.vector.tensor_tensor(
                    out=qbz[:, PAD:], in0=xqe, in1=xqo, op=AluOpType.add)
                # 6-tap sliding sum over the pair domain (DVE scan),
                # extended 6 steps so initial is just KAPPA:
                #   sh6n[t] = sum(qb[t-5..t]) + KAPPA   (valid for t >= 5)
                # old-style R6 starting at pair m == sh6n[m+5].
                sh6 = r6_pool.tile([P, NQI + PAD], F16, tag="sh6")
                nc.vector.tensor_tensor_scan(
                    out=sh6[:, 0:NQI],
                    data0=qbz[:, PAD:PAD + NQI],
                    data1=qbz[:, 0:NQI],
                    initial=KAPPA,
                    op0=AluOpType.add, op1=AluOpType.subtract)

                # presubtract (DVE fp16 2x, full-width contiguous):
                #   we[j] = sh6n[j+5] - xqe[j]   (= R6[j] - x_even[j])
                #   wo[j] = sh6n[j+6] - xqo[j+6] (= R6[j+1] - x_odd[j+6])
                # junk at j%262 >= 256 is never consumed by the matmuls.
                wq = wq_pool.tile([P, 2 * NQI], F16, tag="wq")
                we = wq[:, 0:NQI]
                wo = wq[:, NQI:2 * NQI]
                nc.vector.tensor_tensor(
                    out=we, in0=sh6[:, 5:5 + NQI], in1=xq[:, 0:NQI],
                    op=AluOpType.subtract)
                nc.vector.tensor_tensor(
                    out=wo, in0=sh6[:, 6:6 + NQI],
                    in1=xq[:, NQI + PAD:2 * NQI + PAD],
                    op=AluOpType.subtract)

                # vertical band matmuls: positive band on W planes, then
                # -121*I on the compare-x slices so PSUM holds
                # z = box - 121*C - 121*x
                load_wts()
                ps = psum.tile([P, NB * W], F32, tag="ps")
                for b in range(NB):
                    lo = max(b - 1, 0)
                    hi = min(b + 1, NB - 1)
                    srcs = []
                    for t in range(lo, hi + 1):
                        wmat = MAIN[b] if t == b else \
                               (WDN if t == b - 1 else WUP)
                        srcs.append((t, wmat))
                    for pl in range(2):
                        psl = ps[:, W * b + MQ * pl:W * b + MQ * (pl + 1)]
                        wpl = we if pl == 0 else wo
                        i = 0
                        for t, wmat in srcs:
                            nc.tensor.matmul(psl, wmat,
                                             wpl[:, NQ * t:NQ * t + MQ],
                                             start=(i == 0), stop=False)
                            i += 1
                        xoff = (0 if pl == 0 else NQI) + NQ * b + 3
                        nc.tensor.matmul(psl, DIAG,
                                         xq[:, xoff:xoff + MQ],
                                         start=False, stop=True)
                live[im] = (ps,)

            def compare(im, ps, ot, slot):
                """Sign(z) -> u8 into slot of a 4-image store buffer."""
                nc.scalar.activation(
                    ot[:, slot * NB * W:(slot + 1) * NB * W],
                    ps[:], ACT_SIGN, bias=0.0, scale=1.0)

            def store_batch(i0, n, eng):
                ot = otbufs.pop(i0)
                dst = out_ext[i0:i0 + n].rearrange(
                    "i (b p) w -> p i b w", p=P)
                eng.dma_start(
                    dst, ot[:, 0:n * NB * W].rearrange(
                        "p (i b w) -> p i b w", i=n, b=NB))

            def epilogue(im):
                (ps,) = live.pop(im)
                if im < 12:
                    i0 = im - im % 4
                    if im % 4 == 0:
                        otbufs[i0] = ot_pool.tile(
                            [P, 4 * NB * W], U8, tag="ot4", name="ot4")
                    compare(im, ps, otbufs[i0], im % 4)
                    if im % 4 == 3:
                        store_batch(i0, 4, nc.gpsimd)
                else:
                    # tail: per-image stores to keep the drain short
                    otbufs[im] = ot_pool.tile(
                        [P, 4 * NB * W], U8, tag="ot4", name="ot1")
                    compare(im, ps, otbufs[im], 0)
                    store_batch(im, 1,
                                nc.gpsimd if im < 14 else nc.sync)

            for im in range(NIMG):
                front(im)
                if im >= 1:
                    epilogue(im - 1)
            epilogue(NIMG - 1)

    nc.compile()
    return nc


def _get_nc():
    if "nc" not in _CACHE:
        _CACHE["nc"] = _build()
        _CACHE["wts"] = _band_weights()
    return _CACHE["nc"]


def postprocess(outputs) -> np.ndarray:
    """Device output is u8 with each row stored plane-major
    [evens(256) | odds(256)]; re-interleave and cast to f32."""
    raw = np.concatenate(list(outputs), axis=0)          # [B,H,W] u8
    v = raw.reshape(B, H, 2, W // 2)
    out = np.empty((B, H, W), dtype=np.float32)
    out[:, :, 0::2] = v[:, :, 0]
    out[:, :, 1::2] = v[:, :, 1]
    return out


def kernel(input_batch: np.ndarray) -> np.ndarray:
    from concourse.bass_utils import run_bass_kernel_spmd

    nc = _get_nc()
    wts = _CACHE["wts"]
    assert input_batch.shape == (B, H, W)
    x = np.ascontiguousarray(input_batch, dtype=np.float32)
    in_maps = [
        {"input": x[c * NIMG:(c + 1) * NIMG], "wts": wts}
        for c in range(NCORES)
    ]
    res = run_bass_kernel_spmd(nc, in_maps, core_ids=list(range(NCORES)))
    return postprocess(r["output"] for r in res.results)


if __name__ == "__main__":
    rng = np.random.default_rng(0)
    x = rng.random((B, H, W), dtype=np.float32)
    y = kernel(x)
    print(y.shape, y.dtype, y.mean())
